# revision 11
# baseline (speedup 1.0000x reference)
"""Trainium2 Bass kernel for nn_Channel_Seq_Big_Attention (v2: sharded w_out).

Reference computation (per batch b of 8):
  x: (N=128, M=8, D=512) tokens; q = x@w_q, k,v = x@w_kv (INNER=512, H=8, DH=64)
  sim[i,j,m,z] = q[i,m]·k[j,z] * DH**-0.5     (cross-seq, cross-modality)
  attn = softmax over the QUERY-seq dim i
  out[i,z,d] = sum_{j,m} attn[i,j,m,z] v[j,m,d]
  y = out.reshape(N, M*H*DH) @ w_out + b_out   (row order z*H*DH + h*DH + d)

v2 layout across the 8 cores:
  - Attention is data-parallel over batch (core c owns batch c), identical to
    the tuned v1 dataflow: S^T tiles per (head-pair, z) -> exp -> segmented
    row sums -> reciprocal -> v*Linv -> PV accumulate, PV lagging the sim
    chain by 3 z-steps.
  - The out projection is TENSOR-parallel over w_out columns: core c holds
    only w_out[:, 512c:512(c+1)] (4MB instead of 32MB -> no more streaming
    the full 32MB weight through every core's DMA, which was the v1 wall).
    Core c's 512 columns are exactly modality m=c of the final y.
  - After each head pair g, its 128 contraction channels (z-major, 256KB
    bf16) bounce SBUF->DRAM and AllGather across the 8 cores (ncfw/SDMA
    silicon, ~20us processing + ~15us wake -> too slow to consume mid-pair).
    All 32 y-projection jobs (group g, batch b) therefore run as a dense
    PE tail in group order: gathers complete while earlier groups' matmuls
    run, so no job waits.  Each core produces y[b, i, 512 cols] for ALL
    batches; the host concatenates slices.
  - Attention pairs pace at the slowest engine (~2us/z-step): sim_psum has
    3 bufs so sim(s+1) needn't wait for exp(s); L sums/reciprocals are bf16
    (DVE 2x_1P packed mode needs all-2B dtypes); the v*Linv scaling splits
    GpSimd/Vector.
"""

import sys

import numpy as np

for _p in ("/opt/trn_rl_repo",):
    if _p not in sys.path:
        sys.path.insert(0, _p)

import ml_dtypes  # noqa: E402

B, N, M, D = 8, 128, 8, 512
H, DH = 8, 64
INNER = H * DH          # 512
T = N * M               # 1024 tokens per batch element
CD = INNER * M          # 4096 contraction dim of out projection
NCORES = 8
NPAIR = H // 2          # 4 head pairs
YJOBS = NPAIR * B       # 32 y-projection jobs (group, batch)

BF16 = ml_dtypes.bfloat16

_CACHE = {}


def build_nc():
    import concourse.bass as bass
    import concourse.mybir as mybir
    import concourse.tile as tile
    from concourse import bacc

    fp32 = mybir.dt.float32
    bf16 = mybir.dt.bfloat16

    nc = bacc.Bacc(trn_type="TRN2", target_bir_lowering=False, debug=False)

    xT = nc.dram_tensor("xT", (D, T), bf16, kind="ExternalInput").ap()
    w_q = nc.dram_tensor("w_q", (D, INNER), bf16, kind="ExternalInput").ap()
    w_k = nc.dram_tensor("w_k", (D, INNER), bf16, kind="ExternalInput").ap()
    w_v = nc.dram_tensor("w_v", (D, INNER), bf16, kind="ExternalInput").ap()
    # per-core column slice of w_out (this core's 512 output columns)
    w_out = nc.dram_tensor("w_out", (CD, INNER), bf16, kind="ExternalInput").ap()
    y = nc.dram_tensor("y", (B, N, INNER), fp32, kind="ExternalOutput").ap()

    KC = D // 128        # 4 contraction chunks for the projections
    PC = INNER // 128    # 4 partition chunks of qT/kT
    SCALE = DH ** -0.5
    NKC = CD // 128      # 32 contraction chunks of the out projection
    PV_LAG = 3
    RG = [list(range(NCORES))]

    with tile.TileContext(nc) as tc:
        with (
            tc.tile_pool(name="persist", bufs=1) as persist,
            tc.tile_pool(name="dram", bufs=1, space="DRAM") as dram,
        ):
            qT_sb = persist.tile([128, PC, T], bf16)
            kT_sb = persist.tile([128, PC, T], bf16)
            v_sb = persist.tile([128, M, INNER], bf16)
            y_sb = persist.tile([128, B, INNER], fp32)
            xT_sb = persist.tile([128, KC, T], bf16)
            wq_sb = persist.tile([128, KC, INNER], bf16)
            wk_sb = persist.tile([128, KC, INNER], bf16)
            wv_sb = persist.tile([128, KC, INNER], bf16)
            wo_sb = persist.tile([128, NKC, INNER], bf16)

            cc_in = dram.tile([NPAIR, 128, M, N], bf16)
            cc_outs = [
                dram.tile([NCORES, 128, M, N], bf16, addr_space="Shared",
                          name=f"cc_out{g}")
                for g in range(NPAIR)
            ]

            for kc in range(KC):
                nc.sync.dma_start(wq_sb[:, kc, :], w_q[kc * 128:(kc + 1) * 128, :])
                nc.sync.dma_start(xT_sb[:, kc, :], xT[kc * 128:(kc + 1) * 128, :])
            for kc in range(KC):
                nc.sync.dma_start(wk_sb[:, kc, :], w_k[kc * 128:(kc + 1) * 128, :])
            for kc in range(KC):
                nc.sync.dma_start(wv_sb[:, kc, :], w_v[kc * 128:(kc + 1) * 128, :])
            # this core's w_out slice: 4MB, prefetched once (no streaming)
            for kc in range(NKC):
                nc.sync.dma_start(wo_sb[:, kc, :],
                                  w_out[kc * 128:(kc + 1) * 128, :])

            # ---- PE warm-up ----
            # Runtime preamble + input DMA take ~8us before the first
            # projection matmul can run; the PE HAM clock-gate needs ~3.4us
            # of sustained activity to reach full clock.  Burn the dead time
            # with matmuls on a zeroed scratch tile (no input dependency).
            warm_sb = persist.tile([128, 512], bf16)
            nc.vector.memset(warm_sb[:], 0.0)

            def emit_warm(pool, n, tag="warm"):
                wp = pool.tile([128, 512], fp32, name=tag, tag=tag)
                for _ in range(n):
                    nc.tensor.matmul(
                        wp[:], warm_sb[:, 0:128], warm_sb[:], start=True, stop=True,
                    )

            # warm-up collective: tiny AllGather so ncfw/SDMA are hot
            # before the first real gather (first-call latency is ~12us)
            ccw_in = dram.tile([1, 128], bf16)
            ccw_out = dram.tile([NCORES, 128], bf16, addr_space="Shared")
            nc.gpsimd.dma_start(ccw_in[:], warm_sb[0:1, 0:128])
            nc.gpsimd.collective_compute(
                "AllGather",
                mybir.AluOpType.bypass,
                replica_groups=RG,
                ins=[ccw_in[:]],
                outs=[ccw_out[:]],
            )

            # ---- projections: qT/kT ((h dh) on partitions, tokens free), v ----
            with tc.tile_pool(name="warm_psum", bufs=1, space="PSUM") as warm_psum:
                emit_warm(warm_psum, 8)

            with tc.tile_pool(name="proj_psum", bufs=4, space="PSUM") as proj_psum:
                for dst, w_sb in ((qT_sb, wq_sb), (kT_sb, wk_sb)):
                    for pc in range(PC):
                        for th in range(T // 512):
                            pj = proj_psum.tile([128, 512], fp32, name="pj", tag="pj")
                            for kc in range(KC):
                                nc.tensor.matmul(
                                    pj[:],
                                    w_sb[:, kc, pc * 128:(pc + 1) * 128],
                                    xT_sb[:, kc, th * 512:(th + 1) * 512],
                                    start=(kc == 0),
                                    stop=(kc == KC - 1),
                                )
                            nc.scalar.copy(dst[:, pc, th * 512:(th + 1) * 512], pj[:])
                for m in range(M):
                    pj = proj_psum.tile([128, 512], fp32, name="pj", tag="pj")
                    for kc in range(KC):
                        nc.tensor.matmul(
                            pj[:],
                            xT_sb[:, kc, m * 128:(m + 1) * 128],
                            wv_sb[:, kc, :],
                            start=(kc == 0),
                            stop=(kc == KC - 1),
                        )
                    nc.scalar.copy(v_sb[:, m, :], pj[:])

            # ---- attention + interleaved sharded out-projection ----
            # Heads are processed in pairs (2g, 2g+1); the two heads' sim
            # matmuls use K row-groups 0-63 / 64-127 and their PV matmuls use
            # output col-groups 0-63 / 64-127, so they overlap on the PE.
            # y job j = (group j//8, batch j%8) consumed at slot j + YLAG
            # (slot = 8*pair + z); jobs past slot 31 run in the tail.
            with (
                tc.tile_pool(name="sim_psum", bufs=3, space="PSUM") as sim_psum,
                tc.tile_pool(name="pv_psum", bufs=1, space="PSUM") as pv_psum,
                tc.tile_pool(name="p_pool", bufs=4) as p_pool,
                tc.tile_pool(name="vt_pool", bufs=5) as vt_pool,
                tc.tile_pool(name="stat_pool", bufs=10) as stat_pool,
                tc.tile_pool(name="of_pool", bufs=2) as of_pool,
            ):
                for g in range(NPAIR):  # head pairs
                    opv = pv_psum.tile([128, M * 128], fp32, name="opv", tag="opv")
                    pv_queue = []

                    def emit_pv(zz, p_z, vt_z, opv=opv):
                        # one accumulation group per head per z-region (groups
                        # in a PSUM zero region must not interleave start/stop)
                        for hh in range(2):
                            for m in range(M):
                                nc.tensor.matmul(
                                    opv[hh * 64:hh * 64 + 64, bass.ts(zz, 128)],
                                    vt_z[:, m, hh, :],
                                    p_z[:, hh * T + m * 128:hh * T + (m + 1) * 128],
                                    start=(m == 0),
                                    stop=(m == M - 1),
                                )
                    hc = g
                    qh = (qT_sb[0:64, hc, :], qT_sb[64:128, hc, :])
                    kh = (kT_sb[0:64, hc, :], kT_sb[64:128, hc, :])
                    for z in range(M):
                        # S^T_z per head: keys (z,j) on partitions, (m,i)
                        # free.  One PSUM tile per HEAD (both token halves).
                        p_sb = p_pool.tile([128, 2 * T], bf16, name="p_sb", tag="p")
                        ps = [
                            sim_psum.tile([128, T], fp32, name=f"ps{hh}", tag="ps")
                            for hh in range(2)
                        ]
                        # HAM keep-alive: burn one matmul into the fresh sim
                        # tile before the real sims (their start=True clears
                        # it), so the PE never idles a full activity window
                        # during engine-paced steps and stays at 2.4 GHz.
                        nc.tensor.matmul(
                            ps[0][:, 0:512], warm_sb[:, 0:128], warm_sb[:],
                            start=True, stop=True,
                        )
                        for th in range(T // 512):
                            for hh in range(2):
                                nc.tensor.matmul(
                                    ps[hh][:, bass.ts(th, 512)],
                                    kh[hh][:, bass.ts(z, 128)],
                                    qh[hh][:, bass.ts(th, 512)],
                                    start=True, stop=True,
                                )
                        if len(pv_queue) >= PV_LAG:
                            emit_pv(*pv_queue.pop(0))
                        for hh in range(2):
                            nc.scalar.activation(
                                p_sb[:, hh * T:(hh + 1) * T], ps[hh][:],
                                mybir.ActivationFunctionType.Exp, scale=SCALE,
                            )
                        # L[j, (h, m)] = sum_i P^T[j, (h, m, i)]: first a
                        # bf16 halves-add on VectorE (packed-2x eligible:
                        # all-2B, unit stride), then a 64-wide reduce.
                        pv3 = p_sb[:].rearrange("p (hm i) -> p hm i", i=128)
                        phalf = stat_pool.tile(
                            [128, 2 * M, 64], bf16, name="phalf", tag="ph")
                        lsum = stat_pool.tile([128, 2 * M], bf16, name="lsum", tag="ls")
                        with nc.allow_low_precision(
                            reason="L row-sums of O(1) exp terms; bf16 keeps "
                            "the DVE 2x packed path, rel err checked in test"
                        ):
                            nc.vector.tensor_tensor(
                                phalf[:], pv3[:, :, 0:64], pv3[:, :, 64:128],
                                op=mybir.AluOpType.add,
                            )
                            nc.vector.tensor_reduce(
                                lsum[:], phalf[:],
                                axis=mybir.AxisListType.X, op=mybir.AluOpType.add,
                            )
                            linv = stat_pool.tile(
                                [128, 2 * M], bf16, name="linv", tag="li")
                            nc.vector.reciprocal(linv[:], lsum[:])
                        # vt[j, m, h, d] = v[j, m, (pair cols)] * Linv[j, (h, m)]
                        vt = vt_pool.tile([128, M, 2, DH], bf16, name="vt", tag="vt")
                        vsl = v_sb[:, :, g * 128:(g + 1) * 128].rearrange(
                            "p m (h d) -> p m h d", h=2
                        )
                        lbc = (linv[:].rearrange("p (h m) -> p m h", h=2)
                               .unsqueeze(3).broadcast_to((128, M, 2, DH)))
                        nc.gpsimd.tensor_tensor(
                            vt[:], vsl[:], lbc[:],
                            op=mybir.AluOpType.mult,
                        )
                        pv_queue.append((z, p_sb, vt))
                    for pv in pv_queue:  # flush the lagged z's of the pair
                        emit_pv(*pv)
                    pv_queue.clear()
                    # opv -> SBUF (bf16), split Scalar/Vector so neither
                    # engine adds a serial bubble at the pair boundary.
                    # both halves on ScalarE: keeps the AllGather issue
                    # chain (of_sb -> bounce -> AG) off VectorE, whose FIFO
                    # can stall on y-path PSUM tiles
                    of_sb = of_pool.tile([128, M, N], bf16, name="of_sb", tag="of")
                    nc.scalar.copy(
                        of_sb[:, 0:4, :],
                        opv[:, 0:512].rearrange("p (z i) -> p z i", i=128),
                    )
                    nc.scalar.copy(
                        of_sb[:, 4:8, :],
                        opv[:, 512:].rearrange("p (z i) -> p z i", i=128),
                    )
                    # bounce to DRAM and AllGather this pair's 128 channels
                    nc.gpsimd.dma_start(cc_in[g], of_sb[:])
                    nc.gpsimd.collective_compute(
                        "AllGather",
                        mybir.AluOpType.bypass,
                        replica_groups=RG,
                        ins=[cc_in[g]],
                        outs=[cc_outs[g][:]],
                    )



            # ---- tail: all 32 y-projection jobs, dense on the PE ----
            # Group order g0..g3: while g0's matmuls run, later gathers
            # finish, so no job stalls on its AllGather.
            with (
                tc.tile_pool(name="yp_psum", bufs=4, space="PSUM") as yp_psum,
                tc.tile_pool(name="ag_pool", bufs=5) as ag_pool,
            ):
                ofa_tiles = {}

                def prefetch_job(j):
                    if not 0 <= j < YJOBS:
                        return
                    g, b = j // 8, j % 8
                    ofa = ag_pool.tile([128, M, N], bf16, name="ofa", tag="ofa")
                    nc.sync.dma_start(ofa[:], cc_outs[g][b])
                    ofa_tiles[j] = ofa

                for j in range(3):
                    prefetch_job(j)
                for j in range(YJOBS):
                    prefetch_job(j + 3)
                    g, b = j // 8, j % 8
                    ofa = ofa_tiles.pop(j)
                    yp = yp_psum.tile([128, 512], fp32, name="yp", tag="yp")
                    for z in range(M):
                        nc.tensor.matmul(
                            yp[:],
                            ofa[:, z, :],
                            wo_sb[:, 4 * z + g, :],
                            start=(z == 0),
                            stop=(z == M - 1),
                        )
                    ysl = y_sb[:, b, :]
                    if g == 0:
                        nc.vector.tensor_copy(ysl, yp[:])
                    else:
                        nc.vector.tensor_tensor(
                            ysl, yp[:], ysl, op=mybir.AluOpType.add)
                    if g == NPAIR - 1:
                        nc.sync.dma_start(y[b], ysl)

    nc.compile()
    return nc


def _get_nc():
    if "nc" not in _CACHE:
        _CACHE["nc"] = build_nc()
    return _CACHE["nc"]


def _host_prep(x, w_q, w_kv, w_out):
    w_k = np.ascontiguousarray(w_kv[:, :INNER]).astype(BF16)
    w_v = np.ascontiguousarray(w_kv[:, INNER:]).astype(BF16)
    wq16 = np.ascontiguousarray(w_q).astype(BF16)
    wo16 = np.ascontiguousarray(w_out).astype(BF16)
    in_maps = []
    for b in range(B):
        # tokens modality-major: (M, N, D) -> (T, D); transpose to (D, T)
        xb = x[b].transpose(1, 0, 2).reshape(T, D)
        xT = np.ascontiguousarray(xb.T).astype(BF16)
        wo_slice = np.ascontiguousarray(wo16[:, b * INNER:(b + 1) * INNER])
        in_maps.append(
            {"xT": xT, "w_q": wq16, "w_k": w_k, "w_v": w_v, "w_out": wo_slice}
        )
    return in_maps


def kernel(x, w_q, w_kv, w_out, b_out):
    from concourse.bass_utils import run_bass_kernel_spmd

    nc = _get_nc()
    in_maps = _host_prep(
        np.asarray(x, np.float32),
        np.asarray(w_q, np.float32),
        np.asarray(w_kv, np.float32),
        np.asarray(w_out, np.float32),
    )
    res = run_bass_kernel_spmd(nc, in_maps, core_ids=list(range(NCORES)))
    # core c returns y[b, i, :] for all b covering output columns
    # [512c, 512(c+1)) == modality m=c; stack cores along the m axis.
    ys = np.stack([res.results[c]["y"] for c in range(NCORES)], axis=2)
    # ys: (B, N, M, D)
    ys = ys + np.asarray(b_out, np.float32).reshape(1, 1, M, D)
    return ys.astype(np.float32)
